# revision 4
# baseline (speedup 1.0000x reference)
"""Trainium2 Bass kernel for 3x3 conv (stride 1, pad 1) + bias.

x [32, 64, 224, 224] f32, weight [128, 64, 3, 3] f32, bias [128] f32
-> out [32, 128, 224, 224] f32.

Data-parallel over 8 NeuronCores: core c computes samples [4c, 4c+4).

Per-core scheme (v4, all dims hardcoded):
- Inputs cast to bf16 on host (PSUM accumulation stays fp32; rel err
  ~3e-3, well inside the gate). bf16 also enables FWL fast weight load.
- The K=128 row-pair packing is built in HBM on the host: xdup holds the
  zero-padded rows on partitions 0-63 and the same rows shifted one row
  up on partitions 64-127. One contiguous 128-partition DMA per strip.
- Per 2-output-row block: 3 K=128 matmuls cover kh=0 (top half) and
  kh=1 (bottom half) for kw=0,1,2. The kh=2 taps are K=64 matmuls
  row-tiled onto the two array halves: even blocks read the top half
  (tile_position (0,0)), odd blocks read the bottom half at a one-slot
  offset (tile_position (64,0)), so two adjacent blocks' kh=2 matmuls
  run CONCURRENTLY in the PE array. Per 2 blocks: 6 full-array slots +
  3 concurrent-pair slots = 9 x 448-cycle slots for 4 output rows =
  the 4.5-slot/block compute roofline (~376us/core intrinsic).
- Strips of 56 output rows (58 padded input rows), triple buffered;
  strip t+1's load is issued before strip t's compute.
- PSUM accumulation; ScalarE evacuates psum->SBUF fused with the bias
  add, writing bf16 (halves store traffic; host upcasts to f32). Store
  tiles batch 8 output rows (~0.46 MB per store DMA).
"""
import numpy as np
import ml_dtypes

import concourse.bass as bass
import concourse.mybir as mybir
import concourse.tile as tile
from concourse import bacc
from concourse.bass_utils import run_bass_kernel_spmd
from concourse._compat import axon_active

N_CORES = 8
S = 4                 # samples per core
IC, OC, H, W = 64, 128, 224, 224
HP, WP = H + 2, W + 2  # padded input dims (226)
QROWS = 56            # output rows per strip
SROWS = QROWS + 2     # 58 padded input rows per strip
NQ = H // QROWS       # 4 strips per sample
BLK = 2               # output rows per block
OBLK = 8              # output rows per store tile (4 blocks)

BF16 = mybir.dt.bfloat16
F32 = mybir.dt.float32
NPBF16 = ml_dtypes.bfloat16


def build_module(repeat=1):
    nc = bacc.Bacc("TRN2", target_bir_lowering=False, debug=not axon_active(),
                   enable_asserts=True, num_devices=N_CORES)
    # xdup[s, 0:64, r, c]   = Ppad[ic, r, c]    (zero-padded input rows)
    # xdup[s, 64:128, r, c] = Ppad[ic, r+1, c]  (shifted one row up)
    xdup = nc.dram_tensor("xdup", [S, 2 * IC, HP, WP], BF16,
                          kind="ExternalInput").ap()
    # wpair[0:64, kw*128+oc] = w[oc, ic, kh=0, kw]; [64:128, ...] = kh=1
    wpair = nc.dram_tensor("wpair", [2 * IC, 3 * OC], BF16,
                           kind="ExternalInput").ap()
    # wk2dup[0:64, kw*128+oc] = wk2dup[64:128, kw*128+oc] = w[oc, ic, 2, kw]
    wk2dup = nc.dram_tensor("wk2dup", [2 * IC, 3 * OC], BF16,
                            kind="ExternalInput").ap()
    bias = nc.dram_tensor("bias", [OC, 1], F32, kind="ExternalInput").ap()
    out = nc.dram_tensor("out", [S, OC, H, W], BF16, kind="ExternalOutput").ap()

    NT = S * NQ  # 16 strips, flattened (s, q)

    with tile.TileContext(nc) as tc:
        with tc.tile_pool(name="wp", bufs=1) as wp, \
             tc.tile_pool(name="s0p", bufs=1) as s0p, \
             tc.tile_pool(name="xp", bufs=3) as xp, \
             tc.tile_pool(name="op", bufs=6) as op, \
             tc.tile_pool(name="pp", bufs=2, space="PSUM") as pp:
            wpt = wp.tile([2 * IC, 3 * OC], BF16)
            wk2t = wp.tile([2 * IC, 3 * OC], BF16)
            btile = wp.tile([OC, 1], F32)
            nc.sync.dma_start(out=wpt, in_=wpair)
            nc.sync.dma_start(out=wk2t, in_=wk2dup)
            nc.sync.dma_start(out=btile, in_=bias)

            def load_strip_into(dst, t):
                s, q = divmod(t, NQ)
                nc.scalar.dma_start(
                    out=dst,
                    in_=xdup[s, :, q * QROWS:q * QROWS + SROWS, :])

            def load_strip(t):
                strip = xp.tile([2 * IC, SROWS * WP], BF16, tag="strip")
                sr = strip.rearrange("p (r c) -> p r c", c=WP)
                load_strip_into(sr, t)
                return sr

            # Strip 0 lives in a dedicated tile, loaded once before the
            # repeat loop; each iteration's tail re-loads it for the next
            # iteration so the loop body never starts with a DMA wait
            # (For_i places an all-engine barrier between iterations, so
            # the reload is guaranteed complete).
            s0t = s0p.tile([2 * IC, SROWS * WP], BF16)
            s0r = s0t.rearrange("p (r c) -> p r c", c=WP)
            load_strip_into(s0r, 0)

            def compute(wrap):
                strips = {0: s0r}
                for t in range(NT):
                    s, q = divmod(t, NQ)
                    if t + 1 < NT:
                        strips[t + 1] = load_strip(t + 1)
                    elif wrap:
                        load_strip_into(s0r, 0)
                    sr = strips[t]

                    for g in range(QROWS // OBLK):
                        ot = op.tile([OC, OBLK, W], BF16)
                        psums = [pp.tile([OC, BLK, W], F32, name="ps%d" % bb)
                                 for bb in range(OBLK // BLK)]
                        # Phase 1: kh0 (top) + kh1 (bottom), K=128,
                        # tap-major so each stationary weight is reused
                        # across the 4 psum banks.
                        for kw in range(3):
                            lhsT = wpt[:, kw * OC:(kw + 1) * OC]
                            for bb in range(OBLK // BLK):
                                u = g * OBLK + bb * BLK
                                nc.tensor.matmul(
                                    psums[bb], lhsT,
                                    sr[:, u:u + BLK, kw:kw + W],
                                    start=(kw == 0), stop=False,
                                    skip_group_check=True)
                        # Phase 2: kh=2, K=64 row-tiled. Even banks use
                        # the top array half reading top-half slots
                        # u+2..u+3; odd banks use the bottom array half
                        # reading bottom-half slots u+1..u+2 (same padded
                        # rows). Adjacent even/odd matmuls occupy
                        # disjoint row groups and run concurrently.
                        for kw in range(3):
                            for bb in range(OBLK // BLK):
                                u = g * OBLK + bb * BLK
                                if bb % 2 == 0:
                                    lhsT = wk2t[0:IC, kw * OC:(kw + 1) * OC]
                                    rhs = sr[0:IC, u + 2:u + 2 + BLK,
                                             kw:kw + W]
                                else:
                                    lhsT = wk2t[IC:2 * IC,
                                                kw * OC:(kw + 1) * OC]
                                    rhs = sr[IC:2 * IC, u + 1:u + 1 + BLK,
                                             kw:kw + W]
                                nc.tensor.matmul(
                                    psums[bb], lhsT, rhs,
                                    start=False, stop=(kw == 2),
                                    skip_group_check=True)
                        for bb in range(OBLK // BLK):
                            nc.scalar.activation(
                                ot[:, bb * BLK:(bb + 1) * BLK, :].rearrange(
                                    "p a b -> p (a b)"),
                                psums[bb].rearrange("p a b -> p (a b)"),
                                mybir.ActivationFunctionType.Identity,
                                bias=btile)
                        oh0 = q * QROWS + g * OBLK
                        nc.sync.dma_start(out=out[s, :, oh0:oh0 + OBLK, :],
                                          in_=ot)

            if repeat == 1:
                compute(wrap=False)
            else:
                with tc.For_i(0, repeat, 1):
                    compute(wrap=True)

    nc.compile()
    return nc


def host_prep(weight, bias):
    w = np.asarray(weight, dtype=np.float32)          # [oc, ic, kh, kw]
    wt = np.transpose(w, (1, 3, 0, 2))                # [ic, kw, oc, kh]
    wpair = np.concatenate([wt[:, :, :, 0], wt[:, :, :, 1]], axis=0) \
        .reshape(2 * IC, 3 * OC).astype(NPBF16)
    wk2 = np.ascontiguousarray(wt[:, :, :, 2]).reshape(IC, 3 * OC)
    wk2dup = np.concatenate([wk2, wk2], axis=0).astype(NPBF16)
    b = np.asarray(bias, dtype=np.float32).reshape(OC, 1)
    return wpair, wk2dup, b


def pad_x(x):
    """[N, 64, 224, 224] f32 -> [N, 128, 226, 226] bf16 row-pair dup."""
    n = x.shape[0]
    xb = np.asarray(x, dtype=np.float32).astype(NPBF16)
    xd = np.zeros((n, 2 * IC, HP, WP), NPBF16)
    xd[:, 0:IC, 1:1 + H, 1:1 + W] = xb          # Ppad[ic, r, c]
    xd[:, IC:2 * IC, 0:H, 1:1 + W] = xb         # Ppad[ic, r+1, c]
    return xd


_module_cache = {}


def get_module(repeat=1):
    if repeat not in _module_cache:
        _module_cache[repeat] = build_module(repeat)
    return _module_cache[repeat]


def kernel(x, weight, bias):
    wpair, wk2dup, b = host_prep(weight, bias)
    xd = pad_x(x)
    nc = get_module()
    in_maps = [{"xdup": xd[c * S:(c + 1) * S], "wpair": wpair,
                "wk2dup": wk2dup, "bias": b} for c in range(N_CORES)]
    res = run_bass_kernel_spmd(nc, in_maps, core_ids=list(range(N_CORES)))
    return np.concatenate([res.results[c]["out"] for c in range(N_CORES)],
                          axis=0).astype(np.float32)


# revision 5
# speedup vs baseline: 1.0732x; 1.0732x over previous
"""Trainium2 Bass kernel for 3x3 conv (stride 1, pad 1) + bias.

x [32, 64, 224, 224] f32, weight [128, 64, 3, 3] f32, bias [128] f32
-> out [32, 128, 224, 224] f32.

Data-parallel over 8 NeuronCores: core c computes samples [4c, 4c+4).

Per-core scheme (v6, all dims hardcoded):
- Inputs cast to bf16 on host (PSUM accumulation stays fp32; rel err
  ~3e-3, well inside the gate). bf16 also enables FWL fast weight load.
- Even/odd row-parity packing, built on host with NO duplication:
  xeo[:, 0:64, r, :] = Ppad[ic, 2r, :], xeo[:, 64:128, r, :] =
  Ppad[ic, 2r+1, :]. Input HBM traffic is 1x (26.8 MB/core), half of
  the v4/v5 row-pair layout.
- Even output rows 2m need padded rows (2m, 2m+1, 2m+2) = (top slot m,
  bottom slot m, top slot m+1): one K=128 matmul @ slot m covers
  kh0(top)+kh1(bottom); the kh2 leftover is a K=64 top-half matmul @
  slot m+1. Odd rows 2m+1 need (2m+1, 2m+2, 2m+3) = (bottom m, top
  m+1, bottom m+1): one K=128 matmul @ slot m+1 covers kh1(top)+
  kh2(bottom); the kh0 leftover is a K=64 bottom-half matmul @ slot m.
  The even-row leftovers run on PE row-group 0-63 (tile_position
  (0,0)) and the odd-row leftovers on rows 64-127 ((64,0)), issued
  adjacently so each pair executes CONCURRENTLY: per 4 output rows,
  6 full-array + 3 concurrent-pair slots = the 4.5-slot/2-rows compute
  roofline (~376us/core intrinsic at 2.4 GHz).
- Each psum bank holds 2 same-parity rows; ScalarE evacuates with the
  fused bias add into the interleaved ot tile (bf16), so store DMAs
  write 8 contiguous output rows.
- Strips of 56 output rows = 29 even/odd slots, triple buffered;
  strip 0 lives in a dedicated tile reloaded at each For_i iteration
  tail so the body never starts with a DMA wait.
"""
import numpy as np
import ml_dtypes

import concourse.bass as bass
import concourse.mybir as mybir
import concourse.tile as tile
from concourse import bacc
from concourse.bass_utils import run_bass_kernel_spmd
from concourse._compat import axon_active

N_CORES = 8
S = 4                 # samples per core
IC, OC, H, W = 64, 128, 224, 224
HP, WP = H + 2, W + 2  # padded input dims (226)
XR = HP // 2          # 113 even/odd slot rows in HBM
QROWS = 56            # output rows per strip
SLOTS = QROWS // 2 + 1  # 29 slots per strip
NQ = H // QROWS       # 4 strips per sample
BLK = 2               # output rows per psum bank (same parity)
OBLK = 8              # output rows per store tile (4 psum banks)

BF16 = mybir.dt.bfloat16
F32 = mybir.dt.float32
NPBF16 = ml_dtypes.bfloat16


def build_module(repeat=1):
    nc = bacc.Bacc("TRN2", target_bir_lowering=False, debug=not axon_active(),
                   enable_asserts=True, num_devices=N_CORES)
    # xeo[s, 0:64, r, c] = Ppad[ic, 2r, c]; [64:128] = Ppad[ic, 2r+1, c]
    xeo = nc.dram_tensor("xeo", [S, 2 * IC, XR, WP], BF16,
                         kind="ExternalInput").ap()
    # weven[0:64, kw*128+oc] = w[oc, ic, kh=0, kw]; [64:128] = kh=1
    weven = nc.dram_tensor("weven", [2 * IC, 3 * OC], BF16,
                           kind="ExternalInput").ap()
    # wodd: top = kh=1, bottom = kh=2
    wodd = nc.dram_tensor("wodd", [2 * IC, 3 * OC], BF16,
                          kind="ExternalInput").ap()
    # wleft: top = kh=2 (even-row leftover), bottom = kh=0 (odd leftover)
    wleft = nc.dram_tensor("wleft", [2 * IC, 3 * OC], BF16,
                           kind="ExternalInput").ap()
    bias = nc.dram_tensor("bias", [OC, 1], F32, kind="ExternalInput").ap()
    out = nc.dram_tensor("out", [S, OC, H, W], BF16, kind="ExternalOutput").ap()

    NT = S * NQ  # 16 strips, flattened (s, q)

    with tile.TileContext(nc) as tc:
        with tc.tile_pool(name="wp", bufs=1) as wp, \
             tc.tile_pool(name="s0p", bufs=1) as s0p, \
             tc.tile_pool(name="xp", bufs=3) as xp, \
             tc.tile_pool(name="op", bufs=6) as op, \
             tc.tile_pool(name="pp", bufs=2, space="PSUM") as pp:
            wet = wp.tile([2 * IC, 3 * OC], BF16)
            wot = wp.tile([2 * IC, 3 * OC], BF16)
            wlt = wp.tile([2 * IC, 3 * OC], BF16)
            btile = wp.tile([OC, 1], F32)
            nc.sync.dma_start(out=wet, in_=weven)
            nc.sync.dma_start(out=wot, in_=wodd)
            nc.sync.dma_start(out=wlt, in_=wleft)
            nc.sync.dma_start(out=btile, in_=bias)

            def load_strip_into(dst, t):
                s, q = divmod(t, NQ)
                nc.scalar.dma_start(
                    out=dst,
                    in_=xeo[s, :, q * (QROWS // 2):q * (QROWS // 2) + SLOTS, :])

            def load_strip(t):
                strip = xp.tile([2 * IC, SLOTS * WP], BF16, tag="strip")
                sr = strip.rearrange("p (r c) -> p r c", c=WP)
                load_strip_into(sr, t)
                return sr

            # Strip 0 lives in a dedicated tile, loaded once before the
            # repeat loop; each iteration's tail re-loads it for the next
            # iteration (For_i's all-engine barrier guarantees completion).
            s0t = s0p.tile([2 * IC, SLOTS * WP], BF16)
            s0r = s0t.rearrange("p (r c) -> p r c", c=WP)
            load_strip_into(s0r, 0)

            def compute(wrap):
                strips = {0: s0r}
                for t in range(NT):
                    s, q = divmod(t, NQ)
                    if t + 1 < NT:
                        strips[t + 1] = load_strip(t + 1)
                    elif wrap:
                        load_strip_into(s0r, 0)
                    sr = strips[t]

                    for g in range(QROWS // OBLK):
                        ot = op.tile([OC, OBLK, W], BF16)
                        otv = ot.rearrange("p (m e) c -> p m e c", e=2)
                        psums = [pp.tile([OC, BLK, W], F32, name="ps%d" % bb)
                                 for bb in range(4)]
                        # psum bb: 0=E0 rows(0,2) 1=O0 rows(1,3)
                        #          2=E1 rows(4,6) 3=O1 rows(5,7)
                        # E_k full @ slots (me, me+1); O_k @ (me+1, me+2)
                        # with me = 4g + 2k.
                        for kw in range(3):
                            for bb in range(4):
                                k, odd = divmod(bb, 2)
                                me = 4 * g + 2 * k + odd
                                lhsT = (wot if odd else wet)[
                                    :, kw * OC:(kw + 1) * OC]
                                nc.tensor.matmul(
                                    psums[bb], lhsT,
                                    sr[:, me:me + 2, kw:kw + W],
                                    start=(kw == 0), stop=False,
                                    skip_group_check=True)
                        # Leftovers: even kh2 on rows 0-63 @ slots
                        # (me+1, me+2); odd kh0 on rows 64-127 @ slots
                        # (me, me+1). A/B adjacent -> concurrent.
                        for kw in range(3):
                            for bb in range(4):
                                k, odd = divmod(bb, 2)
                                me = 4 * g + 2 * k
                                if odd:
                                    lhsT = wlt[IC:2 * IC,
                                               kw * OC:(kw + 1) * OC]
                                    rhs = sr[IC:2 * IC, me:me + 2,
                                             kw:kw + W]
                                else:
                                    lhsT = wlt[0:IC, kw * OC:(kw + 1) * OC]
                                    rhs = sr[0:IC, me + 1:me + 3,
                                             kw:kw + W]
                                nc.tensor.matmul(
                                    psums[bb], lhsT, rhs,
                                    start=False, stop=(kw == 2),
                                    skip_group_check=True)
                        for bb in range(4):
                            k, odd = divmod(bb, 2)
                            nc.scalar.activation(
                                otv[:, 2 * k:2 * k + 2, odd, :],
                                psums[bb],
                                mybir.ActivationFunctionType.Identity,
                                bias=btile)
                        oh0 = q * QROWS + g * OBLK
                        nc.sync.dma_start(out=out[s, :, oh0:oh0 + OBLK, :],
                                          in_=ot)

            if repeat == 1:
                compute(wrap=False)
            else:
                with tc.For_i(0, repeat, 1):
                    compute(wrap=True)

    nc.compile()
    return nc


def host_prep(weight, bias):
    w = np.asarray(weight, dtype=np.float32)          # [oc, ic, kh, kw]
    wt = np.transpose(w, (1, 3, 0, 2))                # [ic, kw, oc, kh]

    def pack(top_kh, bot_kh):
        return np.concatenate([wt[:, :, :, top_kh], wt[:, :, :, bot_kh]],
                              axis=0).reshape(2 * IC, 3 * OC).astype(NPBF16)

    weven = pack(0, 1)
    wodd = pack(1, 2)
    wleft = pack(2, 0)
    b = np.asarray(bias, dtype=np.float32).reshape(OC, 1)
    return weven, wodd, wleft, b


def pad_x(x):
    """[N, 64, 224, 224] f32 -> [N, 128, 113, 226] bf16 even/odd rows."""
    n = x.shape[0]
    xb = np.asarray(x, dtype=np.float32).astype(NPBF16)
    pp = np.zeros((n, IC, HP, WP), NPBF16)
    pp[:, :, 1:1 + H, 1:1 + W] = xb
    xeo = np.empty((n, 2 * IC, XR, WP), NPBF16)
    xeo[:, 0:IC] = pp[:, :, 0::2, :]
    xeo[:, IC:2 * IC] = pp[:, :, 1::2, :]
    return xeo


def prep(x, weight, bias):
    """Full inputs -> per-core in_maps list."""
    weven, wodd, wleft, b = host_prep(weight, bias)
    xeo = pad_x(x)
    return [{"xeo": xeo[c * S:(c + 1) * S], "weven": weven, "wodd": wodd,
             "wleft": wleft, "bias": b} for c in range(N_CORES)]


_module_cache = {}


def get_module(repeat=1):
    if repeat not in _module_cache:
        _module_cache[repeat] = build_module(repeat)
    return _module_cache[repeat]


def kernel(x, weight, bias):
    in_maps = prep(x, weight, bias)
    nc = get_module()
    res = run_bass_kernel_spmd(nc, in_maps, core_ids=list(range(N_CORES)))
    return np.concatenate([res.results[c]["out"] for c in range(N_CORES)],
                          axis=0).astype(np.float32)
